# revision 5
# baseline (speedup 1.0000x reference)
"""EquivSetConv (hypergraph message passing) Trainium2 Bass kernel — v1.

Math (reference):
  Xd = segment_sum(dif_vals * X[dif_cols], dif_rows, N)
  Xe = segment_sum((Xd@W1+b1)[vertex], edges, E)
  Xv = segment_sum(concat(Xd[vertex], Xe[edges]) @ W2 + b2, vertex, N)
  out = ((1-a)*Xv + a*Xd) @ W + b

Reassociation (exact up to fp):
  A[e]  = segment_sum(concat(Xd,1)[vertex], edges, E)   # 65 cols: feats+count
  Xe    = A @ [W1; b1]                                  # = A@W1 + cnt_e x b1
  B[v]  = segment_sum(Xe[edges], vertex, N)
  out   = cnt_v o (Xd@M1 + 1 x (1-a)(b2@W)) + Xd@(aW) + B@M2 + 1 x b
          where M1 = (1-a) W2top@W, M2 = (1-a) W2bot@W

Distribution: nodes sharded 8 ways by row range; incidence tokens bucketed to
the core owning the destination row. Per-core sparse steps run as: dma_gather
of source rows (256B each) -> one-hot matmul accumulation in PSUM. Slot
schedule packs cells (dest 128-groups) back-to-back with exact max-over-cores
capacity; cells share 128-token tiles at boundaries via masked one-hot
columns (one matmul per cell x tile overlap).

Step-1 accumulates all source buckets of a group in a PSUM-resident block
(G1 split into thirds of <=33 groups). Scatter for steps 2/4 runs in
transposed form (stationary = gathered rows incl. ones column), so A and B
come out feature-major and no transposes are needed in steps 3/5. The only
collective is an AllReduce of A^T [65, EP], split in two halves to overlap
with the tail of step 2 and the Xe dense pass.
"""
import sys
import numpy as np

sys.path.insert(0, "/opt/trn_rl_repo")

D = 64
NC = 8
CHUNK = 1024        # dma_gather tokens per call (SWDGE ring: 1024 desc/queue)
MB = 16             # one-hot columns built per DVE op
ALPHA = 0.5
BUCKET = 32768      # int16 gather index range
LOOKAHEAD = 320     # gather prefetch horizon, in matmul columns
TRACE = True
LAST_EXEC_NS = None
LAST_RESULTS = None


def _wrap16(a):
    a = np.asarray(a, np.int16)
    return np.tile(a.reshape(-1, 16).T, (8, 1))  # [128, T/16]


def _plan(percore_cells, n_cells, seg_of_cell, min_cap, mm_cell_order=None):
    """Slot schedule: cells packed back-to-back inside segments (segment
    starts 128-aligned); capacity = max token count over cores (>= min_cap).
    Matmul columns: one per (cell, overlapped 128-slot tile), emitted in
    mm_cell_order (defaults to cell id order). Cells belonging to one
    accumulation unit must be adjacent in mm_cell_order."""
    cnt = np.zeros((NC, n_cells), np.int64)
    for c in range(NC):
        cnt[c] = np.bincount(percore_cells[c], minlength=n_cells)
    cap = np.maximum(cnt.max(0), min_cap)
    off = np.zeros(n_cells, np.int64)
    pad = (-cap) % 128
    cap = cap + np.where(pad <= 32, pad, 0)  # close cheap tile boundaries
    segs = []  # (seg_id, slot_base, slot_end_padded)
    cur = 0
    i = 0
    while i < n_cells:
        j, base = i, cur
        while j < n_cells and seg_of_cell[j] == seg_of_cell[i]:
            off[j] = cur
            cur += cap[j]
            j += 1
        cur = -(-cur // 128) * 128
        segs.append((int(seg_of_cell[i]), base, cur))
        i = j
    T = cur
    if mm_cell_order is None:
        mm_cell_order = range(n_cells)
    mm_tile, mm_cell = [], []
    mbase = np.full(n_cells, -1, np.int64)
    for cix in mm_cell_order:
        if cap[cix] == 0:
            continue
        t0 = off[cix] // 128
        t1 = (off[cix] + cap[cix] - 1) // 128
        mbase[cix] = len(mm_tile)
        for t in range(t0, t1 + 1):
            mm_tile.append(t)
            mm_cell.append(cix)
    calls = []  # (seg_id, slot_base, n_tokens)
    tile2call = {}
    for sid, base, end in segs:
        o = base
        while o < end:
            n = min(CHUNK, end - o)
            for t in range(o // 128, (o + n) // 128):
                tile2call[t] = len(calls)
            calls.append((sid, o, n))
            o += n
    first_need = [len(mm_tile)] * len(calls)
    for m, t in enumerate(mm_tile):
        k = tile2call[t]
        if m < first_need[k]:
            first_need[k] = m
    issue_order = sorted(range(len(calls)), key=lambda k: first_need[k])
    pos_in_issue = {k: i for i, k in enumerate(issue_order)}
    mm_callpos = [pos_in_issue[tile2call[t]] for t in mm_tile]
    return dict(cap=cap, off=off, T=T, segs=segs, calls=calls,
                mm_tile=mm_tile, mm_cell=mm_cell, mbase=mbase,
                M=len(mm_tile), first_need=first_need,
                issue_order=issue_order, mm_callpos=mm_callpos)


def _fill(plan, cells, grel, drel, val=None):
    """Per-core tables: gidx[T] (source row in segment's table), and
    per-matmul-column dest-row / value arrays [M, 128] (-1 / 0 = unused)."""
    T, M = plan["T"], plan["M"]
    off, mbase = plan["off"], plan["mbase"]
    order = np.argsort(cells, kind="stable")
    cs, gs, ds = cells[order], grel[order], drel[order]
    if len(cs):
        starts = np.ones(len(cs), bool)
        starts[1:] = cs[1:] != cs[:-1]
        idx = np.arange(len(cs))
        run = np.zeros(len(cs), np.int64)
        run[starts] = idx[starts]
        run = np.maximum.accumulate(run)
        rank = idx - run
    else:
        rank = np.zeros(0, np.int64)
    assert len(cs) == 0 or (rank < plan["cap"][cs]).all()
    slot = off[cs] + rank
    tile, p = slot // 128, slot % 128
    m = mbase[cs] + (tile - off[cs] // 128)
    gidx = np.zeros(T, np.int64)
    gidx[slot] = gs
    drelM = np.full((M, 128), -1.0, np.float32)
    drelM[m, p] = ds
    out = dict(gidx=_wrap16(gidx))
    import ml_dtypes
    out["drelM"] = np.ascontiguousarray(drelM.astype(ml_dtypes.bfloat16).T)
    if val is not None:
        v = np.zeros(T, np.float32)
        v[slot] = val[order]
        out["val"] = np.ascontiguousarray(v.reshape(-1, 128).T)
    return out


def _prep(inputs, n_edges):
    import ml_dtypes
    X = np.ascontiguousarray(np.asarray(inputs["X"], np.float32))
    N = X.shape[0]
    assert N % NC == 0
    S = N // NC
    G1 = -(-S // 128)
    SP = G1 * 128
    EG = -(-n_edges // 128)
    EP = EG * 128
    NB = -(-N // BUCKET)

    dr = np.asarray(inputs["dif_rows"], np.int64)
    dc = np.asarray(inputs["dif_cols"], np.int64)
    dv = np.asarray(inputs["dif_vals"], np.float32)
    vx = np.asarray(inputs["vertex"], np.int64)
    eg = np.asarray(inputs["edges"], np.int64)
    assert eg.max() < n_edges and vx.max() < N and dr.max() < N and dc.max() < N
    assert EP <= 32768 and SP <= 32768

    # ---- step-1 cells: cell = b*G1 + g (bucket-major slot streams); matmuls
    # run group-major (all buckets of a group adjacent -> one open psum
    # accumulation at a time); unit(cell) = g ----
    n_cells1 = NB * G1
    seg1 = np.arange(n_cells1) // G1
    unit1 = (np.arange(n_cells1) % G1).tolist()
    min1 = np.zeros(n_cells1, np.int64)
    min1[:G1] = 1
    order1 = [b * G1 + g for g in range(G1) for b in range(NB)]

    def core_tok1(c):
        mask = (dr >= c * S) & (dr < (c + 1) * S)
        d = dr[mask] - c * S
        b = dc[mask] // BUCKET
        return b * G1 + d // 128, dc[mask] - b * BUCKET, d % 128, dv[mask]

    tok1 = [core_tok1(c) for c in range(NC)]
    plan1 = _plan([t[0] for t in tok1], n_cells1, seg1, min1, order1)

    # ---- step-2 cells: edge groups (two AllReduce halves) ----
    EGH = -(-EG // 2)
    seg2 = (np.arange(EG) >= EGH).astype(np.int64)

    def core_tok2(c):
        mask = (vx >= c * S) & (vx < (c + 1) * S)
        return eg[mask] // 128, vx[mask] - c * S, eg[mask] % 128

    tok2 = [core_tok2(c) for c in range(NC)]
    plan2 = _plan([t[0] for t in tok2], EG, seg2, np.ones(EG, np.int64))

    # ---- step-4 cells: (source Xe half, node group); matmuls group-major
    # so step-4 gathers from half h can start as soon as Xe half h lands ----
    EH = EGH * 128
    n_cells4 = 2 * G1
    seg4 = np.arange(n_cells4) // G1
    unit4 = list(range(n_cells4))  # each (half, group) accumulates alone
    min4 = np.ones(n_cells4, np.int64)

    def core_tok4(c):
        mask = (vx >= c * S) & (vx < (c + 1) * S)
        v = vx[mask] - c * S
        e = eg[mask]
        h = e // EH
        return h * G1 + v // 128, e - h * EH, v % 128

    tok4 = [core_tok4(c) for c in range(NC)]
    plan4 = _plan([t[0] for t in tok4], n_cells4, seg4, min4)

    # ---- weights ----
    W1 = np.asarray(inputs["W1_w"], np.float32)
    W1b = np.asarray(inputs["W1_b"], np.float32)
    W2 = np.asarray(inputs["W2_w"], np.float32)
    W2b = np.asarray(inputs["W2_b"], np.float32)
    Wf = np.asarray(inputs["W_w"], np.float32)
    Wb = np.asarray(inputs["W_b"], np.float32)
    a = ALPHA
    w1ext = np.concatenate([W1, W1b[None, :]], 0)
    m1ext = np.concatenate([(1 - a) * (W2[:D] @ Wf),
                            ((1 - a) * (W2b @ Wf))[None, :]], 0)
    waext = np.concatenate([a * Wf, Wb[None, :]], 0)
    # step-4 gathers A rows directly (Xe = A_ext @ w1ext is linear, so W1
    # folds past the segment-sum): B @ M2 = C @ (w1ext @ M2), C = sum A_ext
    m2wp = w1ext @ ((1 - a) * (W2[D:] @ Wf))

    Xb = np.zeros((N, 2 * D), ml_dtypes.bfloat16)
    Xb[:, :D] = X
    shared = {
        "Xb": Xb,
        "m1ext": m1ext.astype(ml_dtypes.bfloat16),
        "waext": waext.astype(ml_dtypes.bfloat16),
        "m2wp": m2wp.astype(ml_dtypes.bfloat16),
        "iota": np.tile(np.arange(128, dtype=ml_dtypes.bfloat16), (128, 1)),
        "ident": np.eye(128).astype(ml_dtypes.bfloat16),
        "ones": np.ones((128, 1), np.float32),
    }

    in_maps = []
    for c in range(NC):
        f1 = _fill(plan1, *tok1[c])
        f2 = _fill(plan2, *tok2[c])
        f4 = _fill(plan4, *tok4[c])
        cntv = np.bincount((tok4[c][0] % G1) * 128 + tok4[c][2],
                           minlength=SP).astype(np.float32)
        in_maps.append(dict(
            shared,
            gidx1=f1["gidx"], drelM1=f1["drelM"], val1=f1["val"],
            gidx2=f2["gidx"], drelM2=f2["drelM"],
            gidx4=f4["gidx"], drelM4=f4["drelM"],
            cntv=np.ascontiguousarray(cntv.reshape(-1, 128).T),
        ))

    meta = dict(N=N, S=S, G1=G1, SP=SP, EG=EG, EP=EP, NB=NB, EGH=EGH,
                plan1=plan1, plan2=plan2, plan4=plan4, unit1=unit1,
                unit4=unit4)
    return meta, in_maps


def _build(meta):
    from concourse import bass, bacc, tile, mybir

    f32, i16 = mybir.dt.float32, mybir.dt.int16
    bf16 = mybir.dt.bfloat16
    N, S, G1, SP, EG, EP, NB = (meta[k] for k in
                                ("N", "S", "G1", "SP", "EG", "EP", "NB"))
    EGH = meta["EGH"]
    plan1, plan2, plan4 = meta["plan1"], meta["plan2"], meta["plan4"]
    unit1 = meta["unit1"]
    unit4_dev = meta["unit4"]

    nc = bacc.Bacc("TRN2", target_bir_lowering=False, debug=False,
                   num_devices=NC, num_swdge_queues=4)

    def par(name, shape, dt=f32, out=False):
        return nc.declare_dram_parameter(name, list(shape), dt, isOutput=out)

    Xb = par("Xb", (N, 2 * D), bf16)
    gidx1 = par("gidx1", (128, plan1["T"] // 16), i16)
    drelM1 = par("drelM1", (128, plan1["M"]), bf16)
    val1 = par("val1", (128, plan1["T"] // 128))
    gidx2 = par("gidx2", (128, plan2["T"] // 16), i16)
    drelM2 = par("drelM2", (128, plan2["M"]), bf16)
    gidx4 = par("gidx4", (128, plan4["T"] // 16), i16)
    drelM4 = par("drelM4", (128, plan4["M"]), bf16)
    m1ext = par("m1ext", (D + 1, D), bf16)
    waext = par("waext", (D + 1, D), bf16)
    m2wp = par("m2wp", (D + 1, D), bf16)
    iota = par("iota", (128, 128), bf16)
    ident = par("ident", (128, 128), bf16)
    ones = par("ones", (128, 1))
    cntv = par("cntv", (128, G1))
    OUT = par("OUT", (SP, D), out=True)

    eq = mybir.AluOpType.is_equal
    mult = mybir.AluOpType.mult
    addop = mybir.AluOpType.add

    with tile.TileContext(nc) as tc:
        with (
            tc.tile_pool(name="metap", bufs=1) as metap,
            tc.tile_pool(name="gidxp", bufs=2) as gidxp,
            tc.tile_pool(name="gpool", bufs=8) as gpool,
            tc.tile_pool(name="g2pool", bufs=8) as g2pool,
            tc.tile_pool(name="mpool", bufs=3) as mpool,
            tc.tile_pool(name="stage", bufs=2) as stage,
            tc.tile_pool(name="dram", bufs=1, space="DRAM") as dram,
        ):
            def load(ap_param, shape, nm, dt=f32, pool=metap):
                t = pool.tile(list(shape), dt, name=nm, tag=nm)
                nc.scalar.dma_start(t[:], ap_param[:])
                return t

            iota_t = load(iota, (128, 128), "iota_t", dt=bf16)
            ident_t = load(ident, (128, 128), "ident_t", dt=bf16)
            ones_t = load(ones, (128, 1), "ones_t")
            m1ext_t = load(m1ext, (D + 1, D), "m1ext_t", dt=bf16)
            waext_t = load(waext, (D + 1, D), "waext_t", dt=bf16)
            m2wp_t = load(m2wp, (D + 1, D), "m2wp_t", dt=bf16)
            cntv_t = load(cntv, (128, G1), "cntv_t")
            drelM1_t = load(drelM1, (128, plan1["M"]), "drelM1_t", dt=bf16)
            val1_t = load(val1, (128, plan1["T"] // 128), "val1_t")
            drelM2_t = load(drelM2, (128, plan2["M"]), "drelM2_t", dt=bf16)
            drelM4_t = load(drelM4, (128, plan4["M"]), "drelM4_t", dt=bf16)

            XdT_sb = metap.tile([D + 1, G1, 128], bf16)
            Xd_hbm = dram.tile([SP, 2 * D], bf16)
            ARW = D + 4
            ApT = [dram.tile([EGH * 128, ARW], bf16, name=f"ApT{h}")
                   for h in range(2)]
            AfT = [dram.tile([EGH * 128, ARW], bf16, addr_space="Shared",
                             name=f"AfT{h}") for h in range(2)]
            # SWDGE gathers must source Local dram; bounce the collective
            # output out of the Shared window first
            AfL = [dram.tile([EGH * 128, 2 * D], bf16, name=f"AfL{h}")
                   for h in range(2)]
            qctr = [0]

            def sparse(plan, gidx_par, drel_t, val_t, srcs, unit_of_cell,
                       matmul_fn, evac_fn):
                M = plan["M"]
                mm_tile, mm_cell = plan["mm_tile"], plan["mm_cell"]
                calls = plan["calls"]
                first_m, last_m = {}, {}
                for mi in range(M):
                    u = unit_of_cell[mm_cell[mi]]
                    if u not in first_m:
                        first_m[u] = mi
                    last_m[u] = mi
                gidx_t = gidxp.tile([128, plan["T"] // 16], i16, tag="gidx",
                                    name="gidx_t")
                gcols = plan["T"] // 16
                gq = -(-gcols // 4)
                for qi in range(0, gcols, gq):
                    qe = min(qi + gq, gcols)
                    nc.scalar.dma_start(gidx_t[:, qi:qe], gidx_par[:, qi:qe])
                tile_src = {}
                issue_order, first_need = plan["issue_order"], plan["first_need"]
                nextc = [0]
                g2_of = {}

                def issue_through(m_hi):
                    while (nextc[0] < len(issue_order)
                           and first_need[issue_order[nextc[0]]] <= m_hi):
                        ck = issue_order[nextc[0]]
                        sid, o, n = calls[ck]
                        cols = n // 128
                        gt = gpool.tile([128, CHUNK // 128, 2 * D], bf16,
                                        tag="g", name="gt")
                        nc.gpsimd.dma_gather(
                            gt[:, :cols, :], srcs[sid],
                            gidx_t[:, o // 16:(o + n) // 16],
                            n, n, 2 * D, queue_num=qctr[0] % 4)
                        qctr[0] += 1
                        for i in range(cols):
                            tile_src[o // 128 + i] = (gt, i, ck)
                        nextc[0] += 1

                m_buf, mb_base, mb_next = None, 0, 0
                for m in range(M):
                    if m >= mb_next:
                        issue_through(m + LOOKAHEAD)
                        k = min(MB, M - m)
                        m_buf = mpool.tile([128, MB, 128], bf16, tag="m",
                                           name="m_buf")
                        ib = iota_t[:].unsqueeze(1).broadcast_to([128, k, 128])
                        db = drel_t[:, m:m + k].unsqueeze(2).broadcast_to(
                            [128, k, 128])
                        nc.vector.tensor_tensor(m_buf[:, :k, :], ib, db, eq)
                        mb_base, mb_next = m, m + k
                    u = unit_of_cell[mm_cell[m]]
                    gt, col, ck = tile_src[mm_tile[m]]
                    if val_t is not None:
                        # val-scaled copy, created at first consumption (so
                        # it lands after the one-hot builds it must not block)
                        if ck not in g2_of:
                            _, o, n = calls[ck]
                            cols = n // 128
                            g2 = g2pool.tile([128, CHUNK // 128, D], bf16,
                                             tag="g2", name="g2")
                            vs = val_t[:, o // 128:o // 128 + cols]
                            nc.vector.tensor_mul(
                                g2[:, :cols, :], gt[:, :cols, :D],
                                vs.unsqueeze(2).broadcast_to([128, cols, D]))
                            g2_of[ck] = g2
                        gt = g2_of[ck]
                    matmul_fn(u, m_buf[:, m - mb_base, :], gt, col,
                              m == first_m[u], m == last_m[u])
                    if m == last_m[u]:
                        evac_fn(u)

            # ================= step 1: diffusion =================
            with (
                tc.tile_pool(name="psA1", bufs=3, space="PSUM") as psA1,
                tc.tile_pool(name="psTr", bufs=2, space="PSUM") as psTr,
            ):
                srcs1 = []
                for b in range(NB):
                    rows = min(BUCKET, N - b * BUCKET)
                    srcs1.append(Xb[b * BUCKET:b * BUCKET + rows, :])
                st1 = {"ps": None, "x": None}

                def mm1(u, onehot, gt, col, start, stop):
                    if start:
                        st1["ps"] = psA1.tile([128, D], f32, tag="ps1",
                                              name="ps1")
                    nc.tensor.matmul(st1["ps"][:], onehot,
                                     gt[:, col, :D], start=start, stop=stop)

                def ev1(g):
                    b4 = g % 4
                    if b4 == 0:
                        st1["x"] = stage.tile([128, 4, 128], bf16, tag="xst",
                                              name="xst")
                        nc.scalar.memzero(st1["x"][:, :, D:])
                    xst = st1["x"]
                    nc.scalar.copy(xst[:, b4, :D], st1["ps"][:])
                    nc.scalar.copy(xst[:, b4, D:D + 1], ones_t[:, 0:1])
                    pT = psTr.tile([D + 1, 128], bf16, tag="pT", name="pT")
                    nc.tensor.transpose(pT[:], xst[:, b4, :D + 1], ident_t[:])
                    nc.scalar.copy(XdT_sb[:, g, :], pT[:])
                    if b4 == 3 or g == G1 - 1:
                        nb = b4 + 1
                        nc.sync.dma_start(
                            Xd_hbm[(g - nb + 1) * 128:(g + 1) * 128, :]
                            .rearrange("(b p) f -> p b f", p=128),
                            xst[:, :nb, :])

                sparse(plan1, gidx1, drelM1_t, val1_t, srcs1, unit1,
                       mm1, ev1)

            # ================= step 2: A^T partials + AllReduce ===========
            # AR half0 is emitted a few cells after its inputs complete, so
            # it reaches the (in-order) gpsimd queue head with deps already
            # satisfied and doesn't stall the remaining gather stream.
            with tc.tile_pool(name="psA2", bufs=4, space="PSUM") as psA2:
                st2 = {"p": None, "a": None}
                ar0_cell = EGH - 1 + 25
                ar0_done = [False]

                def emit_ar(h):
                    nc.gpsimd.collective_compute(
                        "AllReduce", addop,
                        replica_groups=[list(range(NC))],
                        ins=[ApT[h].opt()], outs=[AfT[h].opt()])
                    nc.sync.dma_start(AfL[h][:, :ARW], AfT[h][:, :])

                def mm2(u, onehot, gt, col, start, stop):
                    if start:
                        st2["p"] = psA2.tile([128, D + 1], f32, tag="pA",
                                             name="pA")
                    nc.tensor.matmul(st2["p"][:], onehot, gt[:, col, :D + 1],
                                     start=start, stop=stop)

                def ev2(cell):
                    h = 0 if cell < EGH else 1
                    cl = cell - h * EGH
                    b4 = cl % 4
                    if b4 == 0:
                        st2["a"] = stage.tile([128, 4, ARW], bf16,
                                              tag="ast", name="ast")
                        nc.scalar.memzero(st2["a"][:, :, D:])
                    nc.scalar.copy(st2["a"][:, b4, :D + 1], st2["p"][:])
                    last_in_half = (cell == EG - 1) or (cl == EGH - 1)
                    if b4 == 3 or last_in_half:
                        nb = b4 + 1
                        nc.sync.dma_start(
                            ApT[h][(cl - nb + 1) * 128:(cl + 1) * 128, :]
                            .rearrange("(b p) f -> p b f", p=128),
                            st2["a"][:, :nb, :])
                    if cell == ar0_cell and cell < EG - 1:
                        emit_ar(0)
                        ar0_done[0] = True

                sparse(plan2, gidx2, drelM2_t, None, [Xd_hbm[:, :]] * 2,
                       list(range(EG)), mm2, ev2)
                if not ar0_done[0]:
                    emit_ar(0)
                emit_ar(1)

            # ====== step 4 + fused output: C = segsum(A_ext), out rows ======
            # Cells run half-major: all A-half-0 groups first (their gathers
            # start right after AllReduce half 0, overlapping AR half 1);
            # C half-0 partials staged in SBUF; half-1 cells finish.
            C_sb = metap.tile([D + 1, G1, 128], bf16)
            with (
                tc.tile_pool(name="psB", bufs=2, space="PSUM") as psB,
                tc.tile_pool(name="psO4", bufs=2, space="PSUM") as psO4,
            ):
                st4 = {"p": None, "o": None}

                def mm4(u, onehot, gt, col, start, stop):
                    if start:
                        st4["p"] = psB.tile([D + 1, 128], f32, tag="pB",
                                            name="pB")
                    nc.tensor.matmul(st4["p"][:], gt[:, col, :D + 1], onehot,
                                     start=start, stop=stop)

                def ev4(u):
                    h, g = divmod(u, G1)
                    if h == 0:
                        nc.scalar.copy(C_sb[:, g, :], st4["p"][:])
                        return
                    bT = stage.tile([D + 1, 128], bf16, tag="bT", name="bT")
                    nc.scalar.copy(bT[:], st4["p"][:])
                    po1 = psO4.tile([128, D], f32, tag="po1", name="po1")
                    nc.tensor.matmul(po1[:], XdT_sb[:, g, :], m1ext_t[:],
                                     start=True, stop=True)
                    poR = psO4.tile([128, D], f32, tag="poR", name="poR")
                    nc.tensor.matmul(poR[:], XdT_sb[:, g, :], waext_t[:],
                                     start=True, stop=False)
                    nc.tensor.matmul(poR[:], C_sb[:, g, :], m2wp_t[:],
                                     start=False, stop=False)
                    nc.tensor.matmul(poR[:], bT[:], m2wp_t[:],
                                     start=False, stop=True)
                    b4 = g % 4
                    if b4 == 0:
                        st4["o"] = stage.tile([128, 4, D], f32, tag="ost",
                                              name="ost")
                    nc.scalar.mul(st4["o"][:, b4, :], po1[:],
                                  cntv_t[:, g:g + 1])
                    nc.vector.tensor_add(st4["o"][:, b4, :],
                                         st4["o"][:, b4, :], poR[:])
                    if b4 == 3 or g == G1 - 1:
                        nb = b4 + 1
                        nc.sync.dma_start(
                            OUT[(g - nb + 1) * 128:(g + 1) * 128, :]
                            .rearrange("(b p) f -> p b f", p=128),
                            st4["o"][:, :nb, :])

                sparse(plan4, gidx4, drelM4_t, None,
                       [AfL[0][:, :], AfL[1][:, :]], unit4_dev,
                       mm4, ev4)

    nc.compile()
    return nc


def _run(inputs, n_edges, sim=False):
    meta, in_maps = _prep(inputs, n_edges)
    nc = _build(meta)
    S, SP = meta["S"], meta["SP"]
    if sim:
        from concourse import bass_interp
        ms = bass_interp.MultiCoreSim(nc, NC, require_finite=False,
                                      require_nnan=False)
        for c in range(NC):
            for k, v in in_maps[c].items():
                ms.cores[c].tensor(k)[:] = v
        ms.simulate()
        outs = [np.array(ms.cores[c].mem_tensor("OUT")).reshape(SP, D)
                for c in range(NC)]
    else:
        from concourse.bass_utils import run_bass_kernel_spmd
        try:
            res = run_bass_kernel_spmd(nc, in_maps, list(range(NC)),
                                       trace=TRACE)
        except ModuleNotFoundError:
            res = run_bass_kernel_spmd(nc, in_maps, list(range(NC)),
                                       trace=False)
        global LAST_EXEC_NS, LAST_RESULTS
        LAST_EXEC_NS = res.exec_time_ns
        LAST_RESULTS = res
        outs = [res.results[c]["OUT"] for c in range(NC)]
    return np.concatenate([o[:S] for o in outs], axis=0).astype(np.float32)


def kernel(**inputs):
    return _run(inputs, 25000, sim=False)



# revision 7
# speedup vs baseline: 1.2437x; 1.2437x over previous
"""EquivSetConv (hypergraph message passing) Trainium2 Bass kernel — v1.

Math (reference):
  Xd = segment_sum(dif_vals * X[dif_cols], dif_rows, N)
  Xe = segment_sum((Xd@W1+b1)[vertex], edges, E)
  Xv = segment_sum(concat(Xd[vertex], Xe[edges]) @ W2 + b2, vertex, N)
  out = ((1-a)*Xv + a*Xd) @ W + b

Reassociation (exact up to fp):
  A[e]  = segment_sum(concat(Xd,1)[vertex], edges, E)   # 65 cols: feats+count
  Xe    = A @ [W1; b1]                                  # = A@W1 + cnt_e x b1
  B[v]  = segment_sum(Xe[edges], vertex, N)
  out   = cnt_v o (Xd@M1 + 1 x (1-a)(b2@W)) + Xd@(aW) + B@M2 + 1 x b
          where M1 = (1-a) W2top@W, M2 = (1-a) W2bot@W

Distribution: nodes sharded 8 ways by row range; incidence tokens bucketed to
the core owning the destination row. Per-core sparse steps run as: dma_gather
of source rows (256B each) -> one-hot matmul accumulation in PSUM. Slot
schedule packs cells (dest 128-groups) back-to-back with exact max-over-cores
capacity; cells share 128-token tiles at boundaries via masked one-hot
columns (one matmul per cell x tile overlap).

Step-1 accumulates all source buckets of a group in a PSUM-resident block
(G1 split into thirds of <=33 groups). Scatter for steps 2/4 runs in
transposed form (stationary = gathered rows incl. ones column), so A and B
come out feature-major and no transposes are needed in steps 3/5. The only
collective is an AllReduce of A rows trimmed to 68 bf16 cols (65 used),
split in two halves to overlap with the tail of step 2; the halves are
bounced into 256B-stride local tables for the step-4 gathers.
"""
import sys
import numpy as np

sys.path.insert(0, "/opt/trn_rl_repo")

D = 64
NC = 8
CHUNK = 1024        # dma_gather tokens per call (SWDGE ring: 1024 desc/queue)
MB = 16             # one-hot columns built per DVE op
ALPHA = 0.5
BUCKET = 32768      # int16 gather index range
LOOKAHEAD = 192     # gather prefetch horizon, in matmul columns
TRACE = True
LAST_EXEC_NS = None
LAST_RESULTS = None


def _wrap16(a):
    a = np.asarray(a, np.int16)
    return np.tile(a.reshape(-1, 16).T, (8, 1))  # [128, T/16]


def _plan(percore_cells, n_cells, seg_of_cell, min_cap, mm_cell_order=None):
    """Slot schedule: cells packed back-to-back inside segments (segment
    starts 128-aligned); capacity = max token count over cores (>= min_cap).
    Matmul columns: one per (cell, overlapped 128-slot tile), emitted in
    mm_cell_order (defaults to cell id order). Cells belonging to one
    accumulation unit must be adjacent in mm_cell_order."""
    cnt = np.zeros((NC, n_cells), np.int64)
    for c in range(NC):
        cnt[c] = np.bincount(percore_cells[c], minlength=n_cells)
    cap = np.maximum(cnt.max(0), min_cap)
    off = np.zeros(n_cells, np.int64)
    pad = (-cap) % 128
    cap = cap + np.where(pad <= 32, pad, 0)  # close cheap tile boundaries
    segs = []  # (seg_id, slot_base, slot_end_padded)
    cur = 0
    i = 0
    while i < n_cells:
        j, base = i, cur
        while j < n_cells and seg_of_cell[j] == seg_of_cell[i]:
            off[j] = cur
            cur += cap[j]
            j += 1
        cur = -(-cur // 128) * 128
        segs.append((int(seg_of_cell[i]), base, cur))
        i = j
    T = cur
    if mm_cell_order is None:
        mm_cell_order = range(n_cells)
    mm_tile, mm_cell = [], []
    mbase = np.full(n_cells, -1, np.int64)
    for cix in mm_cell_order:
        if cap[cix] == 0:
            continue
        t0 = off[cix] // 128
        t1 = (off[cix] + cap[cix] - 1) // 128
        mbase[cix] = len(mm_tile)
        for t in range(t0, t1 + 1):
            mm_tile.append(t)
            mm_cell.append(cix)
    calls = []  # (seg_id, slot_base, n_tokens)
    tile2call = {}
    for sid, base, end in segs:
        o = base
        while o < end:
            n = min(CHUNK, end - o)
            for t in range(o // 128, (o + n) // 128):
                tile2call[t] = len(calls)
            calls.append((sid, o, n))
            o += n
    first_need = [len(mm_tile)] * len(calls)
    for m, t in enumerate(mm_tile):
        k = tile2call[t]
        if m < first_need[k]:
            first_need[k] = m
    issue_order = sorted(range(len(calls)), key=lambda k: first_need[k])
    pos_in_issue = {k: i for i, k in enumerate(issue_order)}
    mm_callpos = [pos_in_issue[tile2call[t]] for t in mm_tile]
    return dict(cap=cap, off=off, T=T, segs=segs, calls=calls,
                mm_tile=mm_tile, mm_cell=mm_cell, mbase=mbase,
                M=len(mm_tile), first_need=first_need,
                issue_order=issue_order, mm_callpos=mm_callpos)


def _fill(plan, cells, grel, drel, val=None):
    """Per-core tables: gidx[T] (source row in segment's table), and
    per-matmul-column dest-row / value arrays [M, 128] (-1 / 0 = unused)."""
    T, M = plan["T"], plan["M"]
    off, mbase = plan["off"], plan["mbase"]
    order = np.argsort(cells, kind="stable")
    cs, gs, ds = cells[order], grel[order], drel[order]
    if len(cs):
        starts = np.ones(len(cs), bool)
        starts[1:] = cs[1:] != cs[:-1]
        idx = np.arange(len(cs))
        run = np.zeros(len(cs), np.int64)
        run[starts] = idx[starts]
        run = np.maximum.accumulate(run)
        rank = idx - run
    else:
        rank = np.zeros(0, np.int64)
    assert len(cs) == 0 or (rank < plan["cap"][cs]).all()
    slot = off[cs] + rank
    tile, p = slot // 128, slot % 128
    m = mbase[cs] + (tile - off[cs] // 128)
    gidx = np.zeros(T, np.int64)
    gidx[slot] = gs
    drelM = np.full((M, 128), -1.0, np.float32)
    drelM[m, p] = ds
    out = dict(gidx=_wrap16(gidx))
    import ml_dtypes
    out["drelM"] = np.ascontiguousarray(drelM.astype(ml_dtypes.bfloat16).T)
    if val is not None:
        v = np.zeros(T, np.float32)
        v[slot] = val[order]
        out["val"] = np.ascontiguousarray(v.reshape(-1, 128).T)
    return out


def _prep(inputs, n_edges):
    import ml_dtypes
    X = np.ascontiguousarray(np.asarray(inputs["X"], np.float32))
    N = X.shape[0]
    assert N % NC == 0
    S = N // NC
    G1 = -(-S // 128)
    SP = G1 * 128
    EG = -(-n_edges // 128)
    EP = EG * 128
    NB = -(-N // BUCKET)

    dr = np.asarray(inputs["dif_rows"], np.int64)
    dc = np.asarray(inputs["dif_cols"], np.int64)
    dv = np.asarray(inputs["dif_vals"], np.float32)
    vx = np.asarray(inputs["vertex"], np.int64)
    eg = np.asarray(inputs["edges"], np.int64)
    assert eg.max() < n_edges and vx.max() < N and dr.max() < N and dc.max() < N
    assert EP <= 32768 and SP <= 32768

    # ---- step-1 cells: cell = b*G1 + g (bucket-major slot streams); matmuls
    # run group-major (all buckets of a group adjacent -> one open psum
    # accumulation at a time); unit(cell) = g ----
    n_cells1 = NB * G1
    seg1 = np.arange(n_cells1) // G1
    unit1 = (np.arange(n_cells1) % G1).tolist()
    min1 = np.zeros(n_cells1, np.int64)
    min1[:G1] = 1
    order1 = [b * G1 + g for g in range(G1) for b in range(NB)]

    def core_tok1(c):
        mask = (dr >= c * S) & (dr < (c + 1) * S)
        d = dr[mask] - c * S
        b = dc[mask] // BUCKET
        return b * G1 + d // 128, dc[mask] - b * BUCKET, d % 128, dv[mask]

    tok1 = [core_tok1(c) for c in range(NC)]
    plan1 = _plan([t[0] for t in tok1], n_cells1, seg1, min1, order1)

    # ---- step-2 cells: edge groups (two AllReduce halves) ----
    EGH = -(-EG // 2)
    seg2 = (np.arange(EG) >= EGH).astype(np.int64)

    def core_tok2(c):
        mask = (vx >= c * S) & (vx < (c + 1) * S)
        return eg[mask] // 128, vx[mask] - c * S, eg[mask] % 128

    tok2 = [core_tok2(c) for c in range(NC)]
    plan2 = _plan([t[0] for t in tok2], EG, seg2, np.ones(EG, np.int64))

    # ---- step-4 cells: (source Xe half, node group); matmuls group-major
    # so step-4 gathers from half h can start as soon as Xe half h lands ----
    EH = EGH * 128
    n_cells4 = 2 * G1
    seg4 = np.arange(n_cells4) // G1
    unit4 = list(range(n_cells4))  # each (half, group) accumulates alone
    min4 = np.ones(n_cells4, np.int64)

    def core_tok4(c):
        mask = (vx >= c * S) & (vx < (c + 1) * S)
        v = vx[mask] - c * S
        e = eg[mask]
        h = e // EH
        return h * G1 + v // 128, e - h * EH, v % 128

    tok4 = [core_tok4(c) for c in range(NC)]
    plan4 = _plan([t[0] for t in tok4], n_cells4, seg4, min4)

    # ---- weights ----
    W1 = np.asarray(inputs["W1_w"], np.float32)
    W1b = np.asarray(inputs["W1_b"], np.float32)
    W2 = np.asarray(inputs["W2_w"], np.float32)
    W2b = np.asarray(inputs["W2_b"], np.float32)
    Wf = np.asarray(inputs["W_w"], np.float32)
    Wb = np.asarray(inputs["W_b"], np.float32)
    a = ALPHA
    w1ext = np.concatenate([W1, W1b[None, :]], 0)
    m1ext = np.concatenate([(1 - a) * (W2[:D] @ Wf),
                            ((1 - a) * (W2b @ Wf))[None, :]], 0)
    waext = np.concatenate([a * Wf, Wb[None, :]], 0)
    # step-4 gathers A rows directly (Xe = A_ext @ w1ext is linear, so W1
    # folds past the segment-sum): B @ M2 = C @ (w1ext @ M2), C = sum A_ext
    m2wp = w1ext @ ((1 - a) * (W2[D:] @ Wf))

    Xb = np.zeros((N, 2 * D), ml_dtypes.bfloat16)
    Xb[:, :D] = X
    shared = {
        "Xb": Xb,
        "m1ext": m1ext.astype(ml_dtypes.bfloat16),
        "waext": waext.astype(ml_dtypes.bfloat16),
        "m2wp": m2wp.astype(ml_dtypes.bfloat16),
        "iota": np.tile(np.arange(128, dtype=ml_dtypes.bfloat16), (128, 1)),
        "ident": np.eye(128).astype(ml_dtypes.bfloat16),
        "ones": np.ones((128, 1), np.float32),
    }

    in_maps = []
    for c in range(NC):
        f1 = _fill(plan1, *tok1[c])
        f2 = _fill(plan2, *tok2[c])
        f4 = _fill(plan4, *tok4[c])
        cntv = np.bincount((tok4[c][0] % G1) * 128 + tok4[c][2],
                           minlength=SP).astype(np.float32)
        in_maps.append(dict(
            shared,
            gidx1=f1["gidx"], drelM1=f1["drelM"], val1=f1["val"],
            gidx2=f2["gidx"], drelM2=f2["drelM"],
            gidx4=f4["gidx"], drelM4=f4["drelM"],
            cntv=np.ascontiguousarray(cntv.reshape(-1, 128).T),
        ))

    meta = dict(N=N, S=S, G1=G1, SP=SP, EG=EG, EP=EP, NB=NB, EGH=EGH,
                plan1=plan1, plan2=plan2, plan4=plan4, unit1=unit1,
                unit4=unit4)
    return meta, in_maps


def _build(meta):
    from concourse import bass, bacc, tile, mybir

    f32, i16 = mybir.dt.float32, mybir.dt.int16
    bf16 = mybir.dt.bfloat16
    N, S, G1, SP, EG, EP, NB = (meta[k] for k in
                                ("N", "S", "G1", "SP", "EG", "EP", "NB"))
    EGH = meta["EGH"]
    plan1, plan2, plan4 = meta["plan1"], meta["plan2"], meta["plan4"]
    unit1 = meta["unit1"]
    unit4_dev = meta["unit4"]

    nc = bacc.Bacc("TRN2", target_bir_lowering=False, debug=False,
                   num_devices=NC, num_swdge_queues=4)

    def par(name, shape, dt=f32, out=False):
        return nc.declare_dram_parameter(name, list(shape), dt, isOutput=out)

    Xb = par("Xb", (N, 2 * D), bf16)
    gidx1 = par("gidx1", (128, plan1["T"] // 16), i16)
    drelM1 = par("drelM1", (128, plan1["M"]), bf16)
    val1 = par("val1", (128, plan1["T"] // 128))
    gidx2 = par("gidx2", (128, plan2["T"] // 16), i16)
    drelM2 = par("drelM2", (128, plan2["M"]), bf16)
    gidx4 = par("gidx4", (128, plan4["T"] // 16), i16)
    drelM4 = par("drelM4", (128, plan4["M"]), bf16)
    m1ext = par("m1ext", (D + 1, D), bf16)
    waext = par("waext", (D + 1, D), bf16)
    m2wp = par("m2wp", (D + 1, D), bf16)
    iota = par("iota", (128, 128), bf16)
    ident = par("ident", (128, 128), bf16)
    ones = par("ones", (128, 1))
    cntv = par("cntv", (128, G1))
    OUT = par("OUT", (SP, D), out=True)

    eq = mybir.AluOpType.is_equal
    mult = mybir.AluOpType.mult
    addop = mybir.AluOpType.add

    with tile.TileContext(nc) as tc:
        with (
            tc.tile_pool(name="metap", bufs=1) as metap,
            tc.tile_pool(name="gidxp", bufs=2) as gidxp,
            tc.tile_pool(name="gpool", bufs=8) as gpool,
            tc.tile_pool(name="g2pool", bufs=8) as g2pool,
            tc.tile_pool(name="mpool", bufs=3) as mpool,
            tc.tile_pool(name="stage", bufs=2) as stage,
            tc.tile_pool(name="dram", bufs=1, space="DRAM") as dram,
        ):
            def load(ap_param, shape, nm, dt=f32, pool=metap):
                t = pool.tile(list(shape), dt, name=nm, tag=nm)
                nc.scalar.dma_start(t[:], ap_param[:])
                return t

            iota_t = load(iota, (128, 128), "iota_t", dt=bf16)
            ident_t = load(ident, (128, 128), "ident_t", dt=bf16)
            ones_t = load(ones, (128, 1), "ones_t")
            m1ext_t = load(m1ext, (D + 1, D), "m1ext_t", dt=bf16)
            waext_t = load(waext, (D + 1, D), "waext_t", dt=bf16)
            m2wp_t = load(m2wp, (D + 1, D), "m2wp_t", dt=bf16)
            cntv_t = load(cntv, (128, G1), "cntv_t")
            drelM1_t = load(drelM1, (128, plan1["M"]), "drelM1_t", dt=bf16)
            val1_t = load(val1, (128, plan1["T"] // 128), "val1_t")
            drelM2_t = load(drelM2, (128, plan2["M"]), "drelM2_t", dt=bf16)
            drelM4_t = load(drelM4, (128, plan4["M"]), "drelM4_t", dt=bf16)

            XdT_sb = metap.tile([D + 1, G1, 128], bf16)
            Xd_hbm = dram.tile([SP, 2 * D], bf16)
            ARW = D + 4
            ApT = [dram.tile([EGH * 128, ARW], bf16, name=f"ApT{h}")
                   for h in range(2)]
            AfT = [dram.tile([EGH * 128, ARW], bf16, addr_space="Shared",
                             name=f"AfT{h}") for h in range(2)]
            # SWDGE gathers must source Local dram; bounce the collective
            # output out of the Shared window first
            AfL = [dram.tile([EGH * 128, 2 * D], bf16, name=f"AfL{h}")
                   for h in range(2)]
            qctr = [0]

            def sparse(plan, gidx_par, drel_t, val_t, srcs, unit_of_cell,
                       matmul_fn, evac_fn):
                M = plan["M"]
                mm_tile, mm_cell = plan["mm_tile"], plan["mm_cell"]
                calls = plan["calls"]
                first_m, last_m = {}, {}
                for mi in range(M):
                    u = unit_of_cell[mm_cell[mi]]
                    if u not in first_m:
                        first_m[u] = mi
                    last_m[u] = mi
                gidx_t = gidxp.tile([128, plan["T"] // 16], i16, tag="gidx",
                                    name="gidx_t")
                gcols = plan["T"] // 16
                gq = -(-gcols // 4)
                for qi in range(0, gcols, gq):
                    qe = min(qi + gq, gcols)
                    nc.scalar.dma_start(gidx_t[:, qi:qe], gidx_par[:, qi:qe])
                tile_src = {}
                issue_order, first_need = plan["issue_order"], plan["first_need"]
                nextc = [0]
                g2_of = {}

                def issue_through(m_hi):
                    while (nextc[0] < len(issue_order)
                           and first_need[issue_order[nextc[0]]] <= m_hi):
                        ck = issue_order[nextc[0]]
                        sid, o, n = calls[ck]
                        cols = n // 128
                        gt = gpool.tile([128, CHUNK // 128, 2 * D], bf16,
                                        tag="g", name="gt")
                        nc.gpsimd.dma_gather(
                            gt[:, :cols, :], srcs[sid],
                            gidx_t[:, o // 16:(o + n) // 16],
                            n, n, 2 * D, queue_num=qctr[0] % 4)
                        qctr[0] += 1
                        for i in range(cols):
                            tile_src[o // 128 + i] = (gt, i, ck)
                        nextc[0] += 1

                m_buf, mb_base, mb_next = None, 0, 0
                for m in range(M):
                    if m >= mb_next:
                        issue_through(m + LOOKAHEAD)
                        k = min(MB, M - m)
                        m_buf = mpool.tile([128, MB, 128], bf16, tag="m",
                                           name="m_buf")
                        ib = iota_t[:].unsqueeze(1).broadcast_to([128, k, 128])
                        db = drel_t[:, m:m + k].unsqueeze(2).broadcast_to(
                            [128, k, 128])
                        nc.vector.tensor_tensor(m_buf[:, :k, :], ib, db, eq)
                        mb_base, mb_next = m, m + k
                    u = unit_of_cell[mm_cell[m]]
                    gt, col, ck = tile_src[mm_tile[m]]
                    if val_t is not None:
                        # val-scaled copy, created at first consumption (so
                        # it lands after the one-hot builds it must not block)
                        if ck not in g2_of:
                            _, o, n = calls[ck]
                            cols = n // 128
                            g2 = g2pool.tile([128, CHUNK // 128, D], bf16,
                                             tag="g2", name="g2")
                            vs = val_t[:, o // 128:o // 128 + cols]
                            nc.vector.tensor_mul(
                                g2[:, :cols, :], gt[:, :cols, :D],
                                vs.unsqueeze(2).broadcast_to([128, cols, D]))
                            g2_of[ck] = g2
                        gt = g2_of[ck]
                    matmul_fn(u, m_buf[:, m - mb_base, :], gt, col,
                              m == first_m[u], m == last_m[u])
                    if m == last_m[u]:
                        evac_fn(u)

            # ================= step 1: diffusion =================
            with (
                tc.tile_pool(name="psA1", bufs=3, space="PSUM") as psA1,
                tc.tile_pool(name="psTr", bufs=2, space="PSUM") as psTr,
            ):
                srcs1 = []
                for b in range(NB):
                    rows = min(BUCKET, N - b * BUCKET)
                    srcs1.append(Xb[b * BUCKET:b * BUCKET + rows, :])
                st1 = {"ps": None, "x": None}

                def mm1(u, onehot, gt, col, start, stop):
                    if start:
                        st1["ps"] = psA1.tile([128, D], f32, tag="ps1",
                                              name="ps1")
                    nc.tensor.matmul(st1["ps"][:], onehot,
                                     gt[:, col, :D], start=start, stop=stop)

                def ev1(g):
                    b4 = g % 4
                    if b4 == 0:
                        st1["x"] = stage.tile([128, 4, 128], bf16, tag="xst",
                                              name="xst")
                        nc.scalar.memzero(st1["x"][:, :, D:])
                    xst = st1["x"]
                    nc.scalar.copy(xst[:, b4, :D], st1["ps"][:])
                    nc.scalar.copy(xst[:, b4, D:D + 1], ones_t[:, 0:1])
                    pT = psTr.tile([D + 1, 128], bf16, tag="pT", name="pT")
                    nc.tensor.transpose(pT[:], xst[:, b4, :D + 1], ident_t[:])
                    nc.scalar.copy(XdT_sb[:, g, :], pT[:])
                    if b4 == 3 or g == G1 - 1:
                        nb = b4 + 1
                        nc.sync.dma_start(
                            Xd_hbm[(g - nb + 1) * 128:(g + 1) * 128, :]
                            .rearrange("(b p) f -> p b f", p=128),
                            xst[:, :nb, :])

                sparse(plan1, gidx1, drelM1_t, val1_t, srcs1, unit1,
                       mm1, ev1)

            # ================= step 2: A^T partials + AllReduce ===========
            # AR half0 is emitted a few cells after its inputs complete, so
            # it reaches the (in-order) gpsimd queue head with deps already
            # satisfied and doesn't stall the remaining gather stream.
            with tc.tile_pool(name="psA2", bufs=4, space="PSUM") as psA2:
                st2 = {"p": None, "a": None}
                ar0_cell = EGH - 1 + 25
                ar0_done = [False]

                def emit_ar(h):
                    nc.gpsimd.collective_compute(
                        "AllReduce", addop,
                        replica_groups=[list(range(NC))],
                        ins=[ApT[h].opt()], outs=[AfT[h].opt()])
                    nc.sync.dma_start(AfL[h][:, :ARW], AfT[h][:, :])

                def mm2(u, onehot, gt, col, start, stop):
                    if start:
                        st2["p"] = psA2.tile([128, D + 1], f32, tag="pA",
                                             name="pA")
                    nc.tensor.matmul(st2["p"][:], onehot, gt[:, col, :D + 1],
                                     start=start, stop=stop)

                def ev2(cell):
                    h = 0 if cell < EGH else 1
                    cl = cell - h * EGH
                    b4 = cl % 4
                    if b4 == 0:
                        st2["a"] = stage.tile([128, 4, ARW], bf16,
                                              tag="ast", name="ast")
                        nc.scalar.memzero(st2["a"][:, :, D:])
                    nc.scalar.copy(st2["a"][:, b4, :D + 1], st2["p"][:])
                    last_in_half = (cell == EG - 1) or (cl == EGH - 1)
                    if b4 == 3 or last_in_half:
                        nb = b4 + 1
                        nc.sync.dma_start(
                            ApT[h][(cl - nb + 1) * 128:(cl + 1) * 128, :]
                            .rearrange("(b p) f -> p b f", p=128),
                            st2["a"][:, :nb, :])
                    if cell == ar0_cell and cell < EG - 1:
                        emit_ar(0)
                        ar0_done[0] = True

                sparse(plan2, gidx2, drelM2_t, None, [Xd_hbm[:, :]] * 2,
                       list(range(EG)), mm2, ev2)
                if not ar0_done[0]:
                    emit_ar(0)
                emit_ar(1)

            # ====== step 4 + fused output: C = segsum(A_ext), out rows ======
            # Cells run half-major: all A-half-0 groups first (their gathers
            # start right after AllReduce half 0, overlapping AR half 1);
            # C half-0 partials staged in SBUF; half-1 cells finish.
            C_sb = metap.tile([D + 1, G1, 128], bf16)
            with (
                tc.tile_pool(name="psB", bufs=2, space="PSUM") as psB,
                tc.tile_pool(name="psO4", bufs=2, space="PSUM") as psO4,
            ):
                st4 = {"p": None, "o": None}

                def mm4(u, onehot, gt, col, start, stop):
                    if start:
                        st4["p"] = psB.tile([D + 1, 128], f32, tag="pB",
                                            name="pB")
                    nc.tensor.matmul(st4["p"][:], gt[:, col, :D + 1], onehot,
                                     start=start, stop=stop)

                def ev4(u):
                    h, g = divmod(u, G1)
                    if h == 0:
                        nc.scalar.copy(C_sb[:, g, :], st4["p"][:])
                        return
                    bT = stage.tile([D + 1, 128], bf16, tag="bT", name="bT")
                    nc.scalar.copy(bT[:], st4["p"][:])
                    po1 = psO4.tile([128, D], f32, tag="po1", name="po1")
                    nc.tensor.matmul(po1[:], XdT_sb[:, g, :], m1ext_t[:],
                                     start=True, stop=True)
                    poR = psO4.tile([128, D], f32, tag="poR", name="poR")
                    nc.tensor.matmul(poR[:], XdT_sb[:, g, :], waext_t[:],
                                     start=True, stop=False)
                    nc.tensor.matmul(poR[:], C_sb[:, g, :], m2wp_t[:],
                                     start=False, stop=False)
                    nc.tensor.matmul(poR[:], bT[:], m2wp_t[:],
                                     start=False, stop=True)
                    b4 = g % 4
                    if b4 == 0:
                        st4["o"] = stage.tile([128, 4, D], f32, tag="ost",
                                              name="ost")
                    nc.scalar.mul(st4["o"][:, b4, :], po1[:],
                                  cntv_t[:, g:g + 1])
                    nc.vector.tensor_add(st4["o"][:, b4, :],
                                         st4["o"][:, b4, :], poR[:])
                    if b4 == 3 or g == G1 - 1:
                        nb = b4 + 1
                        nc.sync.dma_start(
                            OUT[(g - nb + 1) * 128:(g + 1) * 128, :]
                            .rearrange("(b p) f -> p b f", p=128),
                            st4["o"][:, :nb, :])

                sparse(plan4, gidx4, drelM4_t, None,
                       [AfL[0][:, :], AfL[1][:, :]], unit4_dev,
                       mm4, ev4)

    nc.compile()
    return nc


def _run(inputs, n_edges, sim=False):
    meta, in_maps = _prep(inputs, n_edges)
    nc = _build(meta)
    S, SP = meta["S"], meta["SP"]
    if sim:
        from concourse import bass_interp
        ms = bass_interp.MultiCoreSim(nc, NC, require_finite=False,
                                      require_nnan=False)
        for c in range(NC):
            for k, v in in_maps[c].items():
                ms.cores[c].tensor(k)[:] = v
        ms.simulate()
        outs = [np.array(ms.cores[c].mem_tensor("OUT")).reshape(SP, D)
                for c in range(NC)]
    else:
        from concourse.bass_utils import run_bass_kernel_spmd
        try:
            res = run_bass_kernel_spmd(nc, in_maps, list(range(NC)),
                                       trace=TRACE)
        except (ModuleNotFoundError, RuntimeError, OSError):
            # trace infra unavailable/flaky (missing axon hook, NTFF stop
            # failure) -- rerun without tracing; correctness is unaffected
            res = run_bass_kernel_spmd(nc, in_maps, list(range(NC)),
                                       trace=False)
        global LAST_EXEC_NS, LAST_RESULTS
        LAST_EXEC_NS = res.exec_time_ns
        LAST_RESULTS = res
        outs = [res.results[c]["OUT"] for c in range(NC)]
    return np.concatenate([o[:S] for o in outs], axis=0).astype(np.float32)


def kernel(**inputs):
    return _run(inputs, 25000, sim=False)

